# revision 32
# baseline (speedup 1.0000x reference)
"""BYOL loss kernel for Trainium2 (8 NeuronCores, SPMD data-parallel).

loss = 2 - 2 * mean_n( <x_n, t_n> / (||x_n|| * ||t_n||) )   over N=8192 rows, D=512.

Sharding: rows split 1024/core across 8 cores. Each core computes 24 per-row
block reductions ([128,512] -> [128,1]): 8 x.t dots, 8 ||x||^2, 8 ||t||^2.

The HWDGE input stream is the bottleneck (~270-410 GB/s under 8-core HBM
contention); compute merely keeps pace and finishes the last chunk fast.
v10: the host interleaves each core's shard as [block i][row p][x-row, t-row]
so ONE DMA per chunk carries both tensors: 6 DMAs instead of 12, one
completion sem per chunk pair, 128 fully-contiguous 4KB descriptors per
256KB-block (vs 2x128 x 2KB), sequential HBM reads.
  - DVE: affine_mul_reduce (out=(a*b), accum=sum, single pass ~0.61us/block):
         8 xt dots + tt blocks {2,3,4,7}
  - ACT: activation(Square, accum_out) ~0.8us/block: 8 xx + tt {0,1,5,6}
  - last chunk carries only xt_7+tt_7 (DVE) and xx_7 (ACT) -> ~1.3us tail
  - dots stored block-major [P, NT, 3]; blocks 0..6 are DMA'd out early so
    only block 7's 12B/partition ride the final DMA receipt (~2us)
(GpSimd compute is out: TensorScalarPtr is ISA-rejected on Pool and Pool has
no free-axis reduce. tensor_tensor_reduce on DVE hangs this HW stack; the
custom-DVE affine_mul_reduce is production-proven. SWDGE and ACT-ring DMA
both measured slower.)
Host gathers the 8192 per-row stats and takes the mean (trivial all-reduce).
"""

import sys

for _p in ("/opt/trn_rl_repo",):
    if _p not in sys.path:
        sys.path.insert(0, _p)

import numpy as np

import concourse.tile as tile
from concourse import bacc, mybir
from concourse import bass_utils

N, D = 8192, 512
NCORES = 8
N_LOC = N // NCORES          # 1024 rows per core
P = 128                      # partitions
NT = N_LOC // P              # 8 row-blocks of [128, 512] per core
CHUNKS = [1, 1, 2, 2, 1, 1]  # row-blocks per dma_start (small first = early start,
                             # small last = short pipeline drain)
IN_BUFS = len(CHUNKS)        # all chunks in flight: DMA ring never starves
TT_ON_ACT = {0, 1, 5, 6}     # tt blocks on ACT; rest fused on DVE

F32 = mybir.dt.float32


def _build():
    nc = bacc.Bacc("TRN2", target_bir_lowering=False, debug=False, num_devices=NCORES)
    # host-interleaved input: row (i*P + p)*2 + s  ->  block i, partition p,
    # s=0: x row (i*128+p), s=1: t row (i*128+p). One DMA per chunk reads a
    # contiguous region; each partition line is one 4KB x-row+t-row pair.
    xt = nc.dram_tensor("xt", [2 * N_LOC, D], F32, kind="ExternalInput").ap()
    # block-major per-row stats: dots[p, i, :] = (<x,t>, <x,x>, <t,t>) of block i
    out = nc.dram_tensor("dots", [P, NT, 3], F32, kind="ExternalOutput").ap()

    xtr = xt.rearrange("(t p s) d -> p t s d", p=P, s=2)

    assert sum(CHUNKS) == NT

    with tile.TileContext(nc) as tc:
        with (
            tc.tile_pool(name="xin", bufs=IN_BUFS) as xpool,
            tc.tile_pool(name="scratch", bufs=4) as spool,
            tc.tile_pool(name="stats", bufs=1) as stats,
        ):
            dots = stats.tile([P, NT, 3], F32, tag="dots")

            def dve_dot(a, b, acc):
                # single-pass fused multiply + row-sum (custom DVE op,
                # exercised on HW by tile_groupnorm_bwd)
                prod = spool.tile([P, D], F32, tag="dve_prod")
                nc.vector.affine_mul_reduce(
                    out=prod[:], accum_out=acc, in0=a, in1=b,
                    scale=1.0, bias=0.0,
                )

            def act_square(a, acc):
                sq = spool.tile([P, D], F32, tag="sq")
                nc.scalar.activation(
                    sq[:], a, mybir.ActivationFunctionType.Square,
                    accum_out=acc,
                )

            base = 0
            for sz in CHUNKS:
                pair_in = xpool.tile([P, sz, 2, D], F32, tag="xtd")
                nc.sync.dma_start(pair_in[:], xtr[:, base : base + sz, :, :])
                for j in range(sz):
                    i = base + j
                    xa = pair_in[:, j, 0, :]
                    ta = pair_in[:, j, 1, :]
                    dve_dot(xa, ta, dots[:, i, 0:1])        # <x,t> on DVE
                    act_square(xa, dots[:, i, 1:2])         # ||x||^2 on ACT
                    if i in TT_ON_ACT:                      # ||t||^2
                        act_square(ta, dots[:, i, 2:3])
                    else:
                        dve_dot(ta, ta, dots[:, i, 2:3])
                base += sz

            # early out-DMA for blocks 0..6; only block 7's stats ride the
            # final DMA's ~2us completion receipt
            nc.sync.dma_start(out[:, 0 : NT - 1, :], dots[:, 0 : NT - 1, :])
            nc.sync.dma_start(out[:, NT - 1 : NT, :], dots[:, NT - 1 : NT, :])

    nc.finalize()
    return nc


_nc_cache = None


def _get_nc():
    global _nc_cache
    if _nc_cache is None:
        _nc_cache = _build()
    return _nc_cache


def run(x, x_target, **spmd_kwargs):
    """Run the SPMD kernel; returns (loss, BassKernelResults)."""
    x = np.ascontiguousarray(np.asarray(x, dtype=np.float32))
    t = np.ascontiguousarray(np.asarray(x_target, dtype=np.float32))
    assert x.shape == (N, D) and t.shape == (N, D)
    nc = _get_nc()
    in_maps = []
    for c in range(NCORES):
        xs = x[c * N_LOC : (c + 1) * N_LOC].reshape(NT, P, D)
        ts = t[c * N_LOC : (c + 1) * N_LOC].reshape(NT, P, D)
        # [NT, P, 2, D] -> flat row (i*P + p)*2 + s
        inter = np.ascontiguousarray(
            np.stack([xs, ts], axis=2).reshape(2 * N_LOC, D)
        )
        in_maps.append({"xt": inter})
    res = bass_utils.run_bass_kernel_spmd(
        nc, in_maps, core_ids=list(range(NCORES)), **spmd_kwargs
    )
    dots = np.stack([np.asarray(r["dots"]) for r in res.results]).astype(np.float64)
    xt_d = dots[:, :, :, 0]
    xx = dots[:, :, :, 1]
    tt = dots[:, :, :, 2]
    EPS = 1e-8  # matches reference: a / max(||a||, eps) per tensor
    cos = xt_d / (np.maximum(np.sqrt(xx), EPS) * np.maximum(np.sqrt(tt), EPS))
    loss = 2.0 - 2.0 * float(np.mean(cos))
    return np.float32(loss), res


def kernel(x, x_target):
    loss, _ = run(x, x_target)
    return loss


# revision 33
# speedup vs baseline: 1.0945x; 1.0945x over previous
"""BYOL loss kernel for Trainium2 (8 NeuronCores, SPMD data-parallel).

loss = 2 - 2 * mean_n( <x_n, t_n> / (||x_n|| * ||t_n||) )   over N=8192 rows, D=512.

Sharding: rows split 1024/core across 8 cores. Each core computes 24 per-row
block reductions ([128,512] -> [128,1]): 8 x.t dots, 8 ||x||^2, 8 ||t||^2.

The HWDGE input stream is the bottleneck (~270-410 GB/s under 8-core HBM
contention); compute merely keeps pace and finishes the last chunk fast.
v10: the host interleaves each core's shard as [block i][row p][x-row, t-row]
so ONE DMA per chunk carries both tensors: 6 DMAs instead of 12, one
completion sem per chunk pair, 128 fully-contiguous 4KB descriptors per
256KB-block (vs 2x128 x 2KB), sequential HBM reads.
  - DVE: affine_mul_reduce (out=(a*b), accum=sum, single pass ~0.61us/block):
         8 xt dots + tt blocks {2,3,4,7}
  - ACT: activation(Square, accum_out) ~0.8us/block: 8 xx + tt {0,1,5,6}
  - last chunk carries only xt_7+tt_7 (DVE) and xx_7 (ACT) -> ~1.3us tail
  - dots stored block-major [P, NT, 3]; blocks 0..6 are DMA'd out early so
    only block 7's 12B/partition ride the final DMA receipt (~2us)
(GpSimd compute is out: TensorScalarPtr is ISA-rejected on Pool and Pool has
no free-axis reduce. tensor_tensor_reduce on DVE hangs this HW stack; the
custom-DVE affine_mul_reduce is production-proven. SWDGE and ACT-ring DMA
both measured slower.)
Host gathers the 8192 per-row stats and takes the mean (trivial all-reduce).
"""

import sys

for _p in ("/opt/trn_rl_repo",):
    if _p not in sys.path:
        sys.path.insert(0, _p)

import numpy as np

import concourse.tile as tile
from concourse import bacc, mybir
from concourse import bass_utils

N, D = 8192, 512
NCORES = 8
N_LOC = N // NCORES          # 1024 rows per core
P = 128                      # partitions
NT = N_LOC // P              # 8 row-blocks of [128, 512] per core
CHUNKS = [1] * 8             # one block-pair (512KB) per dma_start: finest
                             # completion granularity smooths ACT's mid-stream
                             # double-block bursts; still only 8 input DMAs
IN_BUFS = len(CHUNKS)        # all chunks in flight: DMA ring never starves
TT_ON_ACT = {0, 1, 5, 6}     # tt blocks on ACT; rest fused on DVE

F32 = mybir.dt.float32


def _build():
    nc = bacc.Bacc("TRN2", target_bir_lowering=False, debug=False, num_devices=NCORES)
    # host-interleaved input: row (i*P + p)*2 + s  ->  block i, partition p,
    # s=0: x row (i*128+p), s=1: t row (i*128+p). One DMA per chunk reads a
    # contiguous region; each partition line is one 4KB x-row+t-row pair.
    xt = nc.dram_tensor("xt", [2 * N_LOC, D], F32, kind="ExternalInput").ap()
    # block-major per-row stats: dots[p, i, :] = (<x,t>, <x,x>, <t,t>) of block i
    out = nc.dram_tensor("dots", [P, NT, 3], F32, kind="ExternalOutput").ap()

    xtr = xt.rearrange("(t p s) d -> p t s d", p=P, s=2)

    assert sum(CHUNKS) == NT

    with tile.TileContext(nc) as tc:
        with (
            tc.tile_pool(name="xin", bufs=IN_BUFS) as xpool,
            tc.tile_pool(name="scratch", bufs=4) as spool,
            tc.tile_pool(name="stats", bufs=1) as stats,
        ):
            dots = stats.tile([P, NT, 3], F32, tag="dots")

            def dve_dot(a, b, acc):
                # single-pass fused multiply + row-sum (custom DVE op,
                # exercised on HW by tile_groupnorm_bwd)
                prod = spool.tile([P, D], F32, tag="dve_prod")
                nc.vector.affine_mul_reduce(
                    out=prod[:], accum_out=acc, in0=a, in1=b,
                    scale=1.0, bias=0.0,
                )

            def act_square(a, acc):
                sq = spool.tile([P, D], F32, tag="sq")
                nc.scalar.activation(
                    sq[:], a, mybir.ActivationFunctionType.Square,
                    accum_out=acc,
                )

            base = 0
            for sz in CHUNKS:
                pair_in = xpool.tile([P, sz, 2, D], F32, tag="xtd")
                nc.sync.dma_start(pair_in[:], xtr[:, base : base + sz, :, :])
                for j in range(sz):
                    i = base + j
                    xa = pair_in[:, j, 0, :]
                    ta = pair_in[:, j, 1, :]
                    dve_dot(xa, ta, dots[:, i, 0:1])        # <x,t> on DVE
                    act_square(xa, dots[:, i, 1:2])         # ||x||^2 on ACT
                    if i in TT_ON_ACT:                      # ||t||^2
                        act_square(ta, dots[:, i, 2:3])
                    else:
                        dve_dot(ta, ta, dots[:, i, 2:3])
                base += sz

            # early out-DMA for blocks 0..6; only block 7's stats ride the
            # final DMA's ~2us completion receipt
            nc.sync.dma_start(out[:, 0 : NT - 1, :], dots[:, 0 : NT - 1, :])
            nc.sync.dma_start(out[:, NT - 1 : NT, :], dots[:, NT - 1 : NT, :])

    nc.finalize()
    return nc


_nc_cache = None


def _get_nc():
    global _nc_cache
    if _nc_cache is None:
        _nc_cache = _build()
    return _nc_cache


def run(x, x_target, **spmd_kwargs):
    """Run the SPMD kernel; returns (loss, BassKernelResults)."""
    x = np.ascontiguousarray(np.asarray(x, dtype=np.float32))
    t = np.ascontiguousarray(np.asarray(x_target, dtype=np.float32))
    assert x.shape == (N, D) and t.shape == (N, D)
    nc = _get_nc()
    in_maps = []
    for c in range(NCORES):
        xs = x[c * N_LOC : (c + 1) * N_LOC].reshape(NT, P, D)
        ts = t[c * N_LOC : (c + 1) * N_LOC].reshape(NT, P, D)
        # [NT, P, 2, D] -> flat row (i*P + p)*2 + s
        inter = np.ascontiguousarray(
            np.stack([xs, ts], axis=2).reshape(2 * N_LOC, D)
        )
        in_maps.append({"xt": inter})
    res = bass_utils.run_bass_kernel_spmd(
        nc, in_maps, core_ids=list(range(NCORES)), **spmd_kwargs
    )
    dots = np.stack([np.asarray(r["dots"]) for r in res.results]).astype(np.float64)
    xt_d = dots[:, :, :, 0]
    xx = dots[:, :, :, 1]
    tt = dots[:, :, :, 2]
    EPS = 1e-8  # matches reference: a / max(||a||, eps) per tensor
    cos = xt_d / (np.maximum(np.sqrt(xx), EPS) * np.maximum(np.sqrt(tt), EPS))
    loss = 2.0 - 2.0 * float(np.mean(cos))
    return np.float32(loss), res


def kernel(x, x_target):
    loss, _ = run(x, x_target)
    return loss
